# revision 41
# baseline (speedup 1.0000x reference)
"""CompactCrossAttention TRN2 kernel — tensor-parallel over heads across 8 cores.

Layout strategy (per core c, heads {2c, 2c+1}):
  - Host pre-transposes activations: xqT [H, B*QL], xkvT [H, B*KL] in bf16,
    and slices per-core weight columns/rows.
  - Attention runs HEAD-SEQUENTIAL per batch (4 phases of 32 kv-tiles each).
    Per kv-tile kt: S^T = K_h Q_h^T -> [128 kv, 1024 q] PSUM, exp on the
    scalar engine (softmax max-subtraction skipped; |S*scale| is O(1)),
    P^T bf16 -> AV accumulates O^T + denominator via a ones-column in V.
  - PSUM budget (8 banks): sT ring 2x[128,1024]f32 (4) + one shared
    o-accumulator [128,1024]f32 (2) + work pool 2x[128,512]f32 (2).
    The o tile is reused across the 4 phases: h0 uses rows 0..64 (denom at
    row 64), h1 uses rows 63..127 (denom at row 63, via a [ones|V] column
    order in v_sb) so the normalize multiply is partition-aligned for both
    heads and no ctx relocate DMA is needed.
  - Optional KERNEL_SFP8=1: S matmuls run fp8e4m3 DoubleRow (2x PE rate).
    qT2/kT2 are stored [128, 2, T] fp8 with slice 1 zeroed: the DoubleRow
    pair contracts (real, zero) slices, so no partition remap is needed.
  - Projections (q, kv of the other batch) and the out-projection are
    interleaved into the attention kt loops at ~2-kt granularity to keep PE
    busy; out partials are written bf16 and summed on host (row-parallel
    all-reduce at gather time).
"""

import os
import sys

import numpy as np

for _p in ("/opt/trn_rl_repo",):
    if os.path.isdir(_p) and _p not in sys.path:
        sys.path.insert(0, _p)

B, QL, KL = 2, 1024, 4096
H, NH, HD = 1024, 16, 64
NCORES = 8
TQ, TK = B * QL, B * KL          # 2048, 8192
KT_H = H // 128                  # 8 hidden k-tiles
NKT = KL // 128                  # 32 kv-token tiles per batch
NCH = KL // 512                  # 8 kv chunks per batch

S_FP8 = os.environ.get("KERNEL_SFP8", "1") == "1"

_cache: dict = {}
PHASE_MARKS: list = []


def _mark(nc, name):
    PHASE_MARKS.append((name, nc.next_id()))


def _make_pools(ctx, tc):
    pools = {
        "const": ctx.enter_context(tc.tile_pool(name="const", bufs=1)),
        "hold": ctx.enter_context(tc.tile_pool(name="hold", bufs=1)),
        "kvhold": ctx.enter_context(tc.tile_pool(name="kvhold", bufs=2)),
        "xs": ctx.enter_context(tc.tile_pool(name="xs", bufs=5)),
        "pp": ctx.enter_context(tc.tile_pool(name="pp", bufs=4)),
        "outp": ctx.enter_context(tc.tile_pool(name="outp", bufs=2)),
        "npool": ctx.enter_context(tc.tile_pool(name="npool", bufs=2)),
        "ps_s": ctx.enter_context(tc.tile_pool(name="ps_s", bufs=2, space="PSUM")),
        "ps_o": ctx.enter_context(tc.tile_pool(name="ps_o", bufs=1, space="PSUM")),
        "ps_w": ctx.enter_context(tc.tile_pool(name="ps_w", bufs=2, space="PSUM")),
        "dram": ctx.enter_context(tc.tile_pool(name="dram", bufs=2, space="DRAM")),
    }
    return pools


def _emit(tc, aps, pools):
    import concourse.bass as bass
    from concourse import mybir

    nc = tc.nc
    f32 = mybir.dt.float32
    bf = mybir.dt.bfloat16
    f8 = mybir.dt.float8e4
    P = 128
    Exp = mybir.ActivationFunctionType.Exp
    DR = mybir.MatmulPerfMode.DoubleRow

    xqT, xkvT, wq, wk, wv, wout, out = (
        aps["xqT"], aps["xkvT"], aps["wq"], aps["wk"], aps["wv"],
        aps["wout"], aps["out"],
    )

    const = pools["const"]
    hold = pools["hold"]
    kvhold = pools["kvhold"]
    xs = pools["xs"]
    pp = pools["pp"]
    outp = pools["outp"]
    npool = pools["npool"]
    dram = pools["dram"]
    ps_s = pools["ps_s"]
    ps_o = pools["ps_o"]
    ps_w = pools["ps_w"]

    # ---- constants / weights ------------------------------------------------
    # DMA order is the critical path to the first exp: wq/wk + the first xq
    # and xkv slices must land first; wv follows, wout loads much later (it is
    # first used by the out-projection). Weights arrive host-pretransposed as
    # [p, kt*m] so the transfers are line-contiguous (2KB lines, no RMW
    # penalty).
    wq_sb = const.tile([P, KT_H, P], bf, tag="wq")
    nc.sync.dma_start(out=wq_sb[:], in_=wq.rearrange("p (kt m) -> p kt m", kt=KT_H))
    wk_sb = const.tile([P, KT_H, P], bf, tag="wk")
    nc.sync.dma_start(out=wk_sb[:], in_=wk.rearrange("p (kt m) -> p kt m", kt=KT_H))
    wv_sb = const.tile([P, KT_H, P], bf, tag="wv")
    wout_sb = const.tile([P, H], bf, tag="wout")
    f16 = mybir.dt.float16
    ones_sb = const.tile([1, 64], f16, tag="ones")
    nc.gpsimd.memset(ones_sb[:], 1.0)

    # warm the PE p-state during the ramp DMAs: ~4us of throwaway matmuls so
    # the first real projections run at full clock
    warm_lhs = const.tile([1, 64], bf, tag="warml")
    nc.gpsimd.memset(warm_lhs[:], 0.0)
    warm_rhs = const.tile([1, 512], bf, tag="warmr")
    nc.gpsimd.memset(warm_rhs[:], 0.0)
    for wi in range(6):
        wps = ps_w.tile([64, 512], f32, tag="w", name=f"warm_{wi}")
        nc.tensor.matmul(wps[:], warm_lhs[:], warm_rhs[:], start=True, stop=True)

    ctx_sb = hold.tile([P, TQ], bf, tag="ctx")
    # S_FP8 layouts: slice 0 holds fp8(X); slice 1 holds the Q quantization
    # residual fp8(Q - fp8(Q)) on the Q side and a duplicate of fp8(K) on the
    # K side, so the DoubleRow pair computes
    #   fp8(K)*fp8(Q) + fp8(K)*(Q - fp8(Q)) ~= fp8(K) * Q
    # at zero PE cost. The Q-side error must be the one cancelled: a query's
    # quantization error is constant across all its keys (it acts as a
    # perturbed query and does not average out in the softmax), while per-key
    # errors wash out in the P@V sum -- measured 6.9e-3 vs 1.9e-2 the other
    # way around.
    if S_FP8:
        qT_sb = hold.tile([P, 2, TQ], f8, tag="qT")
    else:
        qT_sb = hold.tile([P, TQ], bf, tag="qT")

    kv_bufs = {}
    for b in range(B):
        if S_FP8:
            kT_b = kvhold.tile([P, 2, KL], f8, tag="kT", name=f"kT_{b}")
        else:
            kT_b = kvhold.tile([P, KL], bf, tag="kT", name=f"kT_{b}")
        v_b = kvhold.tile([P, NKT, 2, 65], bf, tag="v", name=f"v_{b}")
        nc.gpsimd.memset(v_b[:, :, :, 64:65], 1.0)
        kv_bufs[b] = (kT_b, v_b)

    xqT_r = xqT.rearrange("(kt p) t -> p kt t", p=P)
    xkvT_r = xkvT.rearrange("(kt p) t -> p kt t", p=P)

    # ---- work lump generators (~850ns PE granules for gap-filling) ---------
    _qpq = {}

    def qproj_half(qc, half):
        if half == 0:
            xq_t = xs.tile([P, KT_H, 512], bf, tag="x", name=f"xq_{qc}")
            nc.sync.dma_start(out=xq_t[:], in_=xqT_r[:, :, qc * 512:(qc + 1) * 512])
            pq = ps_w.tile([P, 512], f32, tag="w", name=f"pq_{qc}")
            _qpq[qc] = (xq_t, pq)
        else:
            xq_t, pq = _qpq.pop(qc)
        for kt in range(half * 4, half * 4 + 4):
            nc.tensor.matmul(
                pq[:], wq_sb[:, kt, :], xq_t[:, kt, :],
                start=(kt == 0), stop=(kt == KT_H - 1),
            )
        if half == 1:
            if S_FP8:
                nc.vector.tensor_copy(out=qT_sb[:, 0, qc * 512:(qc + 1) * 512], in_=pq[:])
                nc.vector.tensor_sub(out=qT_sb[:, 1, qc * 512:(qc + 1) * 512],
                                     in0=pq[:],
                                     in1=qT_sb[:, 0, qc * 512:(qc + 1) * 512])
            else:
                nc.vector.tensor_copy(out=qT_sb[:, qc * 512:(qc + 1) * 512], in_=pq[:])

    xkv_tiles = {}
    _kpk = {}

    def kv_dma(b, ch):
        xkv_t = xs.tile([P, KT_H, 512], bf, tag="x", name=f"xkv_{b}_{ch}")
        nc.sync.dma_start(
            out=xkv_t[:],
            in_=xkvT_r[:, :, b * KL + ch * 512: b * KL + (ch + 1) * 512],
        )
        xkv_tiles[(b, ch)] = xkv_t

    def kv_k_half(b, ch, half):
        xkv_t = xkv_tiles[(b, ch)]
        if half == 0:
            pk = ps_w.tile([P, 512], f32, tag="w", name=f"pk_{b}_{ch}")
            _kpk[(b, ch)] = pk
        else:
            pk = _kpk.pop((b, ch))
        for kt in range(half * 4, half * 4 + 4):
            nc.tensor.matmul(
                pk[:], wk_sb[:, kt, :], xkv_t[:, kt, :],
                start=(kt == 0), stop=(kt == KT_H - 1),
            )
        if half == 1:
            kT_b = kv_bufs[b][0]
            if S_FP8:
                nc.vector.tensor_copy(out=kT_b[:, 0, ch * 512:(ch + 1) * 512], in_=pk[:])
                nc.vector.tensor_copy(out=kT_b[:, 1, ch * 512:(ch + 1) * 512], in_=pk[:])
            else:
                nc.vector.tensor_copy(out=kT_b[:, ch * 512:(ch + 1) * 512], in_=pk[:])

    def kv_v_half(b, ch, half):
        xkv_t = xkv_tiles[(b, ch)] if half == 0 else xkv_tiles.pop((b, ch))
        v_b = kv_bufs[b][1]
        for mt in range(half * 2, half * 2 + 2):
            pv = ps_w.tile([P, P], f32, tag="w", name=f"pv_{b}_{ch}_{mt}")
            for kt in range(KT_H):
                nc.tensor.matmul(
                    pv[:], xkv_t[:, kt, mt * 128:(mt + 1) * 128], wv_sb[:, kt, :],
                    start=(kt == 0), stop=(kt == KT_H - 1),
                )
            ktile = ch * 4 + mt
            nc.vector.tensor_copy(out=v_b[:, ktile, 0, 0:64], in_=pv[:, 0:64])
            nc.vector.tensor_copy(out=v_b[:, ktile, 1, 0:64], in_=pv[:, 64:128])

    def kv_chunk_lumps(b, ch, dma_next=True):
        lumps = []
        if dma_next and ch + 1 < NCH:
            lumps.append(lambda: kv_dma(b, ch + 1))
        lumps.append(lambda: kv_k_half(b, ch, 0))
        lumps.append(lambda: kv_k_half(b, ch, 1))
        lumps.append(lambda: kv_v_half(b, ch, 0))
        lumps.append(lambda: kv_v_half(b, ch, 1))
        return lumps

    def outproj_tile(b, mt, tail=False):
        tok0 = b * QL + mt * P
        ot = outp.tile([P, H], bf, tag="ot", name=f"ot_{b}_{mt}")
        for nn in range(2):
            po = ps_w.tile([P, 512], f32, tag="w", name=f"po_{b}_{mt}_{nn}")
            nc.tensor.matmul(
                po[:], ctx_sb[:, tok0:tok0 + P], wout_sb[:, nn * 512:(nn + 1) * 512],
                start=True, stop=True,
            )
            if tail and nn == 1:
                nc.scalar.copy(out=ot[:, nn * 512:(nn + 1) * 512], in_=po[:])
            else:
                nc.vector.tensor_copy(out=ot[:, nn * 512:(nn + 1) * 512], in_=po[:])
        nc.sync.dma_start(out=out[tok0:tok0 + P, :], in_=ot[:])

    # ---- attention phase ----------------------------------------------------
    def s_matmul(sT, kT_b, b, h, kt, qcs=(0, 1)):
        for qc in qcs:
            q0 = b * QL + qc * 512
            if S_FP8:
                nc.tensor.matmul(
                    sT[:, qc * 512:(qc + 1) * 512],
                    kT_b[64 * h:64 * (h + 1), :, kt * 128:(kt + 1) * 128],
                    qT_sb[64 * h:64 * (h + 1), :, q0:q0 + 512],
                    start=True, stop=True, perf_mode=DR,
                )
            else:
                nc.tensor.matmul(
                    sT[:, qc * 512:(qc + 1) * 512],
                    kT_b[64 * h:64 * (h + 1), kt * 128:(kt + 1) * 128],
                    qT_sb[64 * h:64 * (h + 1), q0:q0 + 512],
                    start=True, stop=True,
                )

    def attn_phase(b, h, o_ps, lumps, pre_lumps=(), split_first_exp=False):
        _mark(nc, f"attn{b}h{h}")
        kT_b, v_b = kv_bufs[b]
        # lumps: either a dict {kt: [lump,...]} (explicit deadlines -- needed
        # when a lump produces kv data this same phase consumes: S(kt) is
        # pre-issued at iteration kt-1, so chunk ch must be emitted by
        # iteration 4ch-2) or a list (no intra-phase deadline, spread evenly).
        # pre_lumps (the previous phase's normalize) go at kt 0 so their PE
        # bits precede the first AV in PE program order.
        lump_at = {0: list(pre_lumps)}
        if isinstance(lumps, dict):
            for kt, ls in lumps.items():
                lump_at.setdefault(kt, []).extend(ls)
        elif lumps:
            for i, lump in enumerate(lumps):
                lump_at.setdefault(1 + (i * (NKT - 6)) // len(lumps), []).append(lump)
        pT_prev = None

        def av(kt, pT):
            for qc in range(2):
                nc.tensor.matmul(
                    o_ps[0:65, qc * 512:(qc + 1) * 512],
                    v_b[:, kt, h, :],
                    pT[:, qc * 512:(qc + 1) * 512],
                    start=(kt == 0), stop=(kt == NKT - 1),
                )

        # S is pre-issued one kt ahead of its exp so the PE->ACT handoff of
        # tile kt overlaps exp(kt-1) instead of serializing after it (the
        # ring-2 WAR on sT allows exactly one tile of lookahead).
        sTs = {}

        def s_issue(kt, qcs=(0, 1)):
            if kt not in sTs:
                sTs[kt] = ps_s.tile([P, QL], f32, tag="s", name=f"sT_{b}_{h}_{kt}")
            s_matmul(sTs[kt], kT_b, b, h, kt, qcs=qcs)

        pTs = {}
        if split_first_exp:
            # fire the first exp as soon as the qc0 half of S exists --
            # shortens the DMA-serial ramp to the first ACT work
            pTs[0] = pp.tile([P, QL], bf, tag="pT", name=f"pT_{b}_{h}_0")
            s_issue(0, qcs=(0,))
            nc.scalar.activation(out=pTs[0][:, 0:512], in_=sTs[0][:, 0:512],
                                 func=Exp, scale=0.125)
            s_issue(0, qcs=(1,))
            nc.scalar.activation(out=pTs[0][:, 512:1024], in_=sTs[0][:, 512:1024],
                                 func=Exp, scale=0.125)
        else:
            s_issue(0)
        for kt in range(NKT):
            if kt + 1 < NKT:
                s_issue(kt + 1)
            if kt in pTs:
                pT = pTs.pop(kt)
            else:
                pT = pp.tile([P, QL], bf, tag="pT", name=f"pT_{b}_{h}_{kt}")
                nc.scalar.activation(out=pT[:], in_=sTs[kt][:], func=Exp, scale=0.125)
            del sTs[kt]
            if pT_prev is not None:
                av(kt - 1, pT_prev)
            pT_prev = pT
            for lump in lump_at.get(kt, ()):
                lump()
        av(NKT - 1, pT_prev)

    def norm_phase(b, h, o_ps):
        _mark(nc, f"norm{b}h{h}")
        # reciprocal of the denominator row (fp16: 0.05% err on a positive
        # denom), broadcast to 64 partitions with a rank-1 PE matmul
        # (ones[1,64]^T @ recip[1,QL]) -- keeps the normalize chain off the
        # DMA queue.
        f16 = mybir.dt.float16
        recip = npool.tile([1, QL], f16, tag="rc", name=f"rc_{b}_{h}")
        with nc.allow_low_precision(reason="fp16 reciprocal of O(1e3) softmax denom: 0.05% rel err"):
            nc.vector.reciprocal(out=recip[:], in_=o_ps[64:65, :])
        rb_sb = npool.tile([64, QL], f32, tag="rb", name=f"rb_{b}_{h}")
        for hf in range(2):
            rb_ps = ps_w.tile([64, 512], f32, tag="w", name=f"rb_{b}_{h}_{hf}")
            nc.tensor.matmul(
                rb_ps[:], ones_sb[:],
                recip[:, hf * 512:(hf + 1) * 512],
                start=True, stop=True,
            )
            nc.vector.tensor_copy(out=rb_sb[:, hf * 512:(hf + 1) * 512], in_=rb_ps[:])
        if h == 0:
            nc.vector.tensor_mul(
                out=ctx_sb[0:64, b * QL:(b + 1) * QL],
                in0=o_ps[0:64, :], in1=rb_sb[:],
            )
        else:
            ctmp = npool.tile([64, QL], bf, tag="ctmp", name=f"ct_{b}")
            nc.vector.tensor_mul(out=ctmp[:], in0=o_ps[0:64, :], in1=rb_sb[:])
            nc.sync.dma_start(
                out=ctx_sb[64:128, b * QL:(b + 1) * QL], in_=ctmp[:],
            )

    # ---- emission timeline --------------------------------------------------
    # kv chunks pipeline into the phase that first consumes them (attention
    # reads kv tiles in kt order); each chunk's DMA lump runs one chunk ahead
    # of its projection lumps so PE never waits on the transfer. ACT-slack
    # phases (b0h1, b1h1) host the movable work: q-proj of b1, the first kv-b1
    # chunks, and the b0 out-projection.
    _mark(nc, "ramp")
    # First-exp critical path: wq + xq(qc0) -> qproj0, wk + a 256-token xkv
    # sub-chunk -> kT cols 0..255, then S(kt0,qc0)/exp fires (split_first_exp).
    qproj_half(0, 0)
    qproj_half(0, 1)
    xkv0_sub = []
    for sc in range(2):
        xst = xs.tile([P, KT_H, 256], bf, tag="x", name=f"xkv0s{sc}")
        nc.sync.dma_start(out=xst[:], in_=xkvT_r[:, :, sc * 256:(sc + 1) * 256])
        xkv0_sub.append(xst)
        pk = ps_w.tile([P, 256], f32, tag="w", name=f"pk00{sc}")
        for kt in range(KT_H):
            nc.tensor.matmul(
                pk[:], wk_sb[:, kt, :], xst[:, kt, :],
                start=(kt == 0), stop=(kt == KT_H - 1),
            )
        kT0 = kv_bufs[0][0]
        if S_FP8:
            nc.vector.tensor_copy(out=kT0[:, 0, sc * 256:(sc + 1) * 256], in_=pk[:])
            nc.vector.tensor_copy(out=kT0[:, 1, sc * 256:(sc + 1) * 256], in_=pk[:])
        else:
            nc.vector.tensor_copy(out=kT0[:, sc * 256:(sc + 1) * 256], in_=pk[:])
        if sc == 0:
            qproj_half(1, 0)
            qproj_half(1, 1)
            nc.sync.dma_start(
                out=wv_sb[:], in_=wv.rearrange("p (kt m) -> p kt m", kt=KT_H))

    def kv_v_sub(sc):
        xst = xkv0_sub[sc]
        v0 = kv_bufs[0][1]
        for mt in range(2):
            pv = ps_w.tile([P, P], f32, tag="w", name=f"pv00{sc}{mt}")
            for kt in range(KT_H):
                nc.tensor.matmul(
                    pv[:], xst[:, kt, mt * 128:(mt + 1) * 128], wv_sb[:, kt, :],
                    start=(kt == 0), stop=(kt == KT_H - 1),
                )
            ktile = sc * 2 + mt
            nc.vector.tensor_copy(out=v0[:, ktile, 0, 0:64], in_=pv[:, 0:64])
            nc.vector.tensor_copy(out=v0[:, ktile, 1, 0:64], in_=pv[:, 64:128])

    kv_v_sub(0)
    kv_dma(0, 1)

    def kv_sched(b, ch_from, lump_at=None):
        # deadline placement: chunk ch's K finishes by iteration 4ch-3 (< the
        # 4ch-2 limit imposed by the one-ahead S pre-issue), V by 4ch-1
        la = lump_at if lump_at is not None else {}
        for ch in range(ch_from, NCH):
            base = max(4 * (ch - 1) - 2, 0)
            slots = [base, base + 1, base + 2, base + 3]
            ls = []
            if ch + 1 < NCH:
                ls.append(lambda ch=ch: kv_dma(b, ch + 1))
            ls += [lambda ch=ch: kv_k_half(b, ch, 0),
                   lambda ch=ch: kv_k_half(b, ch, 1),
                   lambda ch=ch: kv_v_half(b, ch, 0),
                   lambda ch=ch: kv_v_half(b, ch, 1)]
            la.setdefault(slots[0], []).extend(ls[:-3])
            for s, l in zip(slots[1:], ls[-3:]):
                la.setdefault(s, []).append(l)
        return la

    lumps_b0h0 = kv_sched(0, 1, {0: [lambda: kv_v_sub(1)]})
    lumps_b0h1 = [
        lambda: nc.sync.dma_start(out=wout_sb[:], in_=wout),
        lambda: qproj_half(2, 0), lambda: qproj_half(2, 1),
        lambda: qproj_half(3, 0), lambda: qproj_half(3, 1),
        lambda: kv_dma(1, 0),
        lambda: kv_k_half(1, 0, 0), lambda: kv_k_half(1, 0, 1),
        lambda: kv_v_half(1, 0, 0), lambda: kv_v_half(1, 0, 1),
        lambda: kv_dma(1, 1),
        lambda: kv_k_half(1, 1, 0), lambda: kv_k_half(1, 1, 1),
        lambda: kv_v_half(1, 1, 0), lambda: kv_v_half(1, 1, 1),
        lambda: kv_dma(1, 2),
    ]
    lumps_b1first = kv_sched(1, 2)
    lumps_b1second = [lambda mt=mt: outproj_tile(0, mt) for mt in range(QL // P)]

    # b1 runs h1 before h0 so the final normalize is the direct-write h0 path
    # (no ctx relocate DMA on the tail critical path)
    phases = [(0, 0), (0, 1), (1, 1), (1, 0)]
    phase_lumps = {(0, 0): lumps_b0h0, (0, 1): lumps_b0h1,
                   (1, 1): lumps_b1first, (1, 0): lumps_b1second}

    pre = ()
    o_prev = None
    bh_prev = None
    for b, h in phases:
        o_ps = ps_o.tile([P, QL], f32, tag="o", name=f"o_{b}_{h}")
        attn_phase(b, h, o_ps, phase_lumps[(b, h)], pre_lumps=pre,
                   split_first_exp=(b == 0 and h == 0))
        pre = ((lambda bb=b, hh=h, oo=o_ps: norm_phase(bb, hh, oo)),)
        o_prev, bh_prev = o_ps, (b, h)

    norm_phase(bh_prev[0], bh_prev[1], o_prev)
    _mark(nc, "tail")
    for mt in range(QL // P):
        outproj_tile(1, mt, tail=True)


def _build(reps=1):
    from contextlib import ExitStack

    import concourse.tile as tile
    from concourse import bacc, mybir

    f32 = mybir.dt.float32
    bf = mybir.dt.bfloat16

    nc = bacc.Bacc("TRN2", target_bir_lowering=False, debug=False,
                   num_devices=NCORES)
    aps = {
        "xqT": nc.dram_tensor("xqT", [H, TQ], bf, kind="ExternalInput").ap(),
        "xkvT": nc.dram_tensor("xkvT", [H, TK], bf, kind="ExternalInput").ap(),
        "wq": nc.dram_tensor("wq", [128, H], bf, kind="ExternalInput").ap(),
        "wk": nc.dram_tensor("wk", [128, H], bf, kind="ExternalInput").ap(),
        "wv": nc.dram_tensor("wv", [128, H], bf, kind="ExternalInput").ap(),
        "wout": nc.dram_tensor("wout", [128, H], bf, kind="ExternalInput").ap(),
        "out": nc.dram_tensor("out", [TQ, H], bf, kind="ExternalOutput").ap(),
    }
    with tile.TileContext(nc) as tc:
        with ExitStack() as ctx:
            pools = _make_pools(ctx, tc)
            for _ in range(reps):
                _emit(tc, aps, pools)
    nc.compile()
    return nc


def get_nc(reps=1):
    key = f"nc{reps}"
    if key not in _cache:
        _cache[key] = _build(reps)
    return _cache[key]


def make_in_maps(query, key_value, w_q, w_kv, w_out):
    import ml_dtypes
    cdt = ml_dtypes.bfloat16

    xq = np.asarray(query, np.float32).reshape(TQ, H)
    xkv = np.asarray(key_value, np.float32).reshape(TK, H)
    xqT = np.ascontiguousarray(xq.T).astype(cdt)
    xkvT = np.ascontiguousarray(xkv.T).astype(cdt)
    w_q = np.asarray(w_q, np.float32)
    w_kv = np.asarray(w_kv, np.float32)
    w_out = np.asarray(w_out, np.float32)

    def wprep(w):
        # [H, 128] -> [128p, KT_H*128m]: p is the within-k-tile partition so
        # the device-side DMA is line-contiguous per partition
        return np.ascontiguousarray(
            w.reshape(KT_H, 128, 128).transpose(1, 0, 2).reshape(128, H)
        ).astype(cdt)

    in_maps = []
    for c in range(NCORES):
        sl = slice(c * 128, (c + 1) * 128)
        in_maps.append({
            "xqT": xqT,
            "xkvT": xkvT,
            "wq": wprep(w_q[:, sl]),
            "wk": wprep(w_kv[:, sl]),
            "wv": wprep(w_kv[:, H + c * 128: H + (c + 1) * 128]),
            "wout": np.ascontiguousarray(w_out[sl, :]).astype(cdt),
        })
    return in_maps


LAST_EXEC_NS = None


def _run(in_maps, trace=False):
    global LAST_EXEC_NS
    from concourse import bass_utils

    nc = get_nc()
    res = bass_utils.run_bass_kernel_spmd(
        nc, in_maps, core_ids=list(range(NCORES)), trace=trace,
    )
    if res.exec_time_ns is not None:
        LAST_EXEC_NS = res.exec_time_ns
    return res


def kernel(query, key_value, w_q, w_kv, w_out):
    in_maps = make_in_maps(query, key_value, w_q, w_kv, w_out)
    res = _run(in_maps)
    total = np.zeros((TQ, H), np.float64)
    for c in range(NCORES):
        total += np.asarray(res.results[c]["out"], np.float64)
    return total.reshape(B, QL, H).astype(np.float32)


# revision 49
# speedup vs baseline: 1.1066x; 1.1066x over previous
"""CompactCrossAttention TRN2 kernel — tensor-parallel over heads across 8 cores.

Layout strategy (per core c, heads {2c, 2c+1}):
  - Host pre-transposes activations: xqT [H, B*QL], xkvT [H, B*KL] in bf16,
    and slices per-core weight columns/rows.
  - Attention runs HEAD-SEQUENTIAL per batch (4 phases of 32 kv-tiles each).
    Per kv-tile kt: S^T = K_h Q_h^T -> [128 kv, 1024 q] PSUM, exp on the
    scalar engine (softmax max-subtraction skipped; |S*scale| is O(1)),
    P^T bf16 -> AV accumulates O^T + denominator via a ones-column in V.
  - PSUM budget (8 banks): sT ring 2x[128,1024]f32 (4) + one shared
    o-accumulator [128,1024]f32 (2) + work pool 2x[128,512]f32 (2).
    The o tile is reused across the 4 phases: h0 uses rows 0..64 (denom at
    row 64), h1 uses rows 63..127 (denom at row 63, via a [ones|V] column
    order in v_sb) so the normalize multiply is partition-aligned for both
    heads and no ctx relocate DMA is needed.
  - Optional KERNEL_SFP8=1: S matmuls run fp8e4m3 DoubleRow (2x PE rate).
    qT2/kT2 are stored [128, 2, T] fp8 with slice 1 zeroed: the DoubleRow
    pair contracts (real, zero) slices, so no partition remap is needed.
  - Projections (q, kv of the other batch) and the out-projection are
    interleaved into the attention kt loops at ~2-kt granularity to keep PE
    busy; out partials are written bf16 and summed on host (row-parallel
    all-reduce at gather time).
"""

import os
import sys

import numpy as np

for _p in ("/opt/trn_rl_repo",):
    if os.path.isdir(_p) and _p not in sys.path:
        sys.path.insert(0, _p)

B, QL, KL = 2, 1024, 4096
H, NH, HD = 1024, 16, 64
NCORES = 8
TQ, TK = B * QL, B * KL          # 2048, 8192
KT_H = H // 128                  # 8 hidden k-tiles
NKT = KL // 128                  # 32 kv-token tiles per batch
NCH = KL // 512                  # 8 kv chunks per batch

S_FP8 = os.environ.get("KERNEL_SFP8", "1") == "1"

_cache: dict = {}
PHASE_MARKS: list = []


def _mark(nc, name):
    PHASE_MARKS.append((name, nc.next_id()))


def _make_pools(ctx, tc):
    pools = {
        "const": ctx.enter_context(tc.tile_pool(name="const", bufs=1)),
        "hold": ctx.enter_context(tc.tile_pool(name="hold", bufs=1)),
        "kvhold": ctx.enter_context(tc.tile_pool(name="kvhold", bufs=2)),
        "xs": ctx.enter_context(tc.tile_pool(name="xs", bufs=6)),
        "pp": ctx.enter_context(tc.tile_pool(name="pp", bufs=4)),
        "outp": ctx.enter_context(tc.tile_pool(name="outp", bufs=2)),
        "npool": ctx.enter_context(tc.tile_pool(name="npool", bufs=2)),
        "ps_s": ctx.enter_context(tc.tile_pool(name="ps_s", bufs=2, space="PSUM")),
        "ps_o": ctx.enter_context(tc.tile_pool(name="ps_o", bufs=1, space="PSUM")),
        "ps_w": ctx.enter_context(tc.tile_pool(name="ps_w", bufs=2, space="PSUM")),
        "dram": ctx.enter_context(tc.tile_pool(name="dram", bufs=2, space="DRAM")),
    }
    return pools


def _emit(tc, aps, pools):
    import concourse.bass as bass
    from concourse import mybir

    nc = tc.nc
    f32 = mybir.dt.float32
    bf = mybir.dt.bfloat16
    f8 = mybir.dt.float8e4
    P = 128
    Exp = mybir.ActivationFunctionType.Exp
    DR = mybir.MatmulPerfMode.DoubleRow

    xqT, xkvT, wq, wk, wv, wout, out = (
        aps["xqT"], aps["xkvT"], aps["wq"], aps["wk"], aps["wv"],
        aps["wout"], aps["out"],
    )

    const = pools["const"]
    hold = pools["hold"]
    kvhold = pools["kvhold"]
    xs = pools["xs"]
    pp = pools["pp"]
    outp = pools["outp"]
    npool = pools["npool"]
    dram = pools["dram"]
    ps_s = pools["ps_s"]
    ps_o = pools["ps_o"]
    ps_w = pools["ps_w"]

    # ---- constants / weights ------------------------------------------------
    # DMA order is the critical path to the first exp: wq/wk + the first xq
    # and xkv slices must land first; wv follows, wout loads much later (it is
    # first used by the out-projection). Weights arrive host-pretransposed as
    # [p, kt*m] so the transfers are line-contiguous (2KB lines, no RMW
    # penalty).
    wq_sb = const.tile([P, KT_H, P], bf, tag="wq")
    nc.sync.dma_start(out=wq_sb[:], in_=wq.rearrange("p (kt m) -> p kt m", kt=KT_H))
    wk_sb = const.tile([P, KT_H, P], bf, tag="wk")
    nc.sync.dma_start(out=wk_sb[:], in_=wk.rearrange("p (kt m) -> p kt m", kt=KT_H))
    wv_sb = const.tile([P, KT_H, P], bf, tag="wv")
    wout_sb = const.tile([P, H], bf, tag="wout")
    f16 = mybir.dt.float16
    ones_sb = const.tile([1, 64], f16, tag="ones")
    nc.gpsimd.memset(ones_sb[:], 1.0)

    # warm the PE p-state during the ramp DMAs: ~4us of throwaway matmuls so
    # the first real projections run at full clock
    warm_lhs = const.tile([1, 64], bf, tag="warml")
    nc.gpsimd.memset(warm_lhs[:], 0.0)
    warm_rhs = const.tile([1, 512], bf, tag="warmr")
    nc.gpsimd.memset(warm_rhs[:], 0.0)
    for wi in range(6):
        wps = ps_w.tile([64, 512], f32, tag="w", name=f"warm_{wi}")
        nc.tensor.matmul(wps[:], warm_lhs[:], warm_rhs[:], start=True, stop=True)

    ctx_sb = hold.tile([P, TQ], bf, tag="ctx")
    # S_FP8 layouts: slice 0 holds fp8(X); slice 1 holds the Q quantization
    # residual fp8(Q - fp8(Q)) on the Q side and a duplicate of fp8(K) on the
    # K side, so the DoubleRow pair computes
    #   fp8(K)*fp8(Q) + fp8(K)*(Q - fp8(Q)) ~= fp8(K) * Q
    # at zero PE cost. The Q-side error must be the one cancelled: a query's
    # quantization error is constant across all its keys (it acts as a
    # perturbed query and does not average out in the softmax), while per-key
    # errors wash out in the P@V sum -- measured 6.9e-3 vs 1.9e-2 the other
    # way around.
    if S_FP8:
        qT_sb = hold.tile([P, 2, TQ], f8, tag="qT")
    else:
        qT_sb = hold.tile([P, TQ], bf, tag="qT")

    kv_bufs = {}
    for b in range(B):
        if S_FP8:
            kT_b = kvhold.tile([P, 2, KL], f8, tag="kT", name=f"kT_{b}")
        else:
            kT_b = kvhold.tile([P, KL], bf, tag="kT", name=f"kT_{b}")
        v_b = kvhold.tile([P, NKT, 2, 65], bf, tag="v", name=f"v_{b}")
        nc.gpsimd.memset(v_b[:, :, :, 64:65], 1.0)
        kv_bufs[b] = (kT_b, v_b)

    xqT_r = xqT.rearrange("(kt p) t -> p kt t", p=P)
    xkvT_r = xkvT.rearrange("(kt p) t -> p kt t", p=P)

    # ---- work lump generators (~850ns PE granules for gap-filling) ---------
    _qpq = {}

    def qproj_half(qc, half):
        if half == 0:
            xq_t = xs.tile([P, KT_H, 512], bf, tag="x", name=f"xq_{qc}")
            nc.sync.dma_start(out=xq_t[:], in_=xqT_r[:, :, qc * 512:(qc + 1) * 512])
            pq = ps_w.tile([P, 512], f32, tag="w", name=f"pq_{qc}")
            _qpq[qc] = (xq_t, pq)
        else:
            xq_t, pq = _qpq.pop(qc)
        for kt in range(half * 4, half * 4 + 4):
            nc.tensor.matmul(
                pq[:], wq_sb[:, kt, :], xq_t[:, kt, :],
                start=(kt == 0), stop=(kt == KT_H - 1),
            )
        if half == 1:
            if S_FP8:
                nc.vector.tensor_copy(out=qT_sb[:, 0, qc * 512:(qc + 1) * 512], in_=pq[:])
                nc.vector.tensor_sub(out=qT_sb[:, 1, qc * 512:(qc + 1) * 512],
                                     in0=pq[:],
                                     in1=qT_sb[:, 0, qc * 512:(qc + 1) * 512])
            else:
                nc.vector.tensor_copy(out=qT_sb[:, qc * 512:(qc + 1) * 512], in_=pq[:])

    xkv_tiles = {}
    _kpk = {}

    def kv_dma(b, ch):
        xkv_t = xs.tile([P, KT_H, 512], bf, tag="x", name=f"xkv_{b}_{ch}")
        nc.sync.dma_start(
            out=xkv_t[:],
            in_=xkvT_r[:, :, b * KL + ch * 512: b * KL + (ch + 1) * 512],
        )
        xkv_tiles[(b, ch)] = xkv_t

    def kv_k_half(b, ch, half):
        xkv_t = xkv_tiles[(b, ch)]
        if half == 0:
            pk = ps_w.tile([P, 512], f32, tag="w", name=f"pk_{b}_{ch}")
            _kpk[(b, ch)] = pk
        else:
            pk = _kpk.pop((b, ch))
        for kt in range(half * 4, half * 4 + 4):
            nc.tensor.matmul(
                pk[:], wk_sb[:, kt, :], xkv_t[:, kt, :],
                start=(kt == 0), stop=(kt == KT_H - 1),
            )
        if half == 1:
            kT_b = kv_bufs[b][0]
            if S_FP8:
                nc.vector.tensor_copy(out=kT_b[:, 0, ch * 512:(ch + 1) * 512], in_=pk[:])
                nc.vector.tensor_copy(out=kT_b[:, 1, ch * 512:(ch + 1) * 512], in_=pk[:])
            else:
                nc.vector.tensor_copy(out=kT_b[:, ch * 512:(ch + 1) * 512], in_=pk[:])

    def kv_v_half(b, ch, half):
        xkv_t = xkv_tiles[(b, ch)] if half == 0 else xkv_tiles.pop((b, ch))
        v_b = kv_bufs[b][1]
        for mt in range(half * 2, half * 2 + 2):
            pv = ps_w.tile([P, P], f32, tag="w", name=f"pv_{b}_{ch}_{mt}")
            for kt in range(KT_H):
                nc.tensor.matmul(
                    pv[:], xkv_t[:, kt, mt * 128:(mt + 1) * 128], wv_sb[:, kt, :],
                    start=(kt == 0), stop=(kt == KT_H - 1),
                )
            ktile = ch * 4 + mt
            nc.vector.tensor_copy(out=v_b[:, ktile, 0, 0:64], in_=pv[:, 0:64])
            nc.vector.tensor_copy(out=v_b[:, ktile, 1, 0:64], in_=pv[:, 64:128])

    def kv_chunk_lumps(b, ch, dma_next=True):
        lumps = []
        if dma_next and ch + 1 < NCH:
            lumps.append(lambda: kv_dma(b, ch + 1))
        lumps.append(lambda: kv_k_half(b, ch, 0))
        lumps.append(lambda: kv_k_half(b, ch, 1))
        lumps.append(lambda: kv_v_half(b, ch, 0))
        lumps.append(lambda: kv_v_half(b, ch, 1))
        return lumps

    def outproj_tile(b, mt, tail=False):
        tok0 = b * QL + mt * P
        ot = outp.tile([P, H], bf, tag="ot", name=f"ot_{b}_{mt}")
        for nn in range(2):
            po = ps_w.tile([P, 512], f32, tag="w", name=f"po_{b}_{mt}_{nn}")
            nc.tensor.matmul(
                po[:], ctx_sb[:, tok0:tok0 + P], wout_sb[:, nn * 512:(nn + 1) * 512],
                start=True, stop=True,
            )
            if tail and nn == 1:
                nc.scalar.copy(out=ot[:, nn * 512:(nn + 1) * 512], in_=po[:])
            else:
                nc.vector.tensor_copy(out=ot[:, nn * 512:(nn + 1) * 512], in_=po[:])
        nc.sync.dma_start(out=out[tok0:tok0 + P, :], in_=ot[:])

    # ---- attention phase ----------------------------------------------------
    def s_matmul(sT, kT_b, b, h, kt, qcs=(0, 1)):
        for qc in qcs:
            q0 = b * QL + qc * 512
            if S_FP8:
                nc.tensor.matmul(
                    sT[:, qc * 512:(qc + 1) * 512],
                    kT_b[64 * h:64 * (h + 1), :, kt * 128:(kt + 1) * 128],
                    qT_sb[64 * h:64 * (h + 1), :, q0:q0 + 512],
                    start=True, stop=True, perf_mode=DR,
                )
            else:
                nc.tensor.matmul(
                    sT[:, qc * 512:(qc + 1) * 512],
                    kT_b[64 * h:64 * (h + 1), kt * 128:(kt + 1) * 128],
                    qT_sb[64 * h:64 * (h + 1), q0:q0 + 512],
                    start=True, stop=True,
                )

    def attn_phase(b, h, o_ps, lumps, pre_lumps=(), split_first_exp=False):
        _mark(nc, f"attn{b}h{h}")
        kT_b, v_b = kv_bufs[b]
        # lumps: either a dict {kt: [lump,...]} (explicit deadlines -- needed
        # when a lump produces kv data this same phase consumes: S(kt) is
        # pre-issued at iteration kt-1, so chunk ch must be emitted by
        # iteration 4ch-2) or a list (no intra-phase deadline, spread evenly).
        # pre_lumps (the previous phase's normalize) go at kt 0 so their PE
        # bits precede the first AV in PE program order.
        lump_at = {0: list(pre_lumps)}
        if isinstance(lumps, dict):
            for kt, ls in lumps.items():
                lump_at.setdefault(kt, []).extend(ls)
        elif lumps:
            for i, lump in enumerate(lumps):
                lump_at.setdefault(1 + (i * (NKT - 6)) // len(lumps), []).append(lump)
        pT_prev = None

        def av(kt, pT):
            for qc in range(2):
                nc.tensor.matmul(
                    o_ps[0:65, qc * 512:(qc + 1) * 512],
                    v_b[:, kt, h, :],
                    pT[:, qc * 512:(qc + 1) * 512],
                    start=(kt == 0), stop=(kt == NKT - 1),
                )

        # S is pre-issued one kt ahead of its exp so the PE->ACT handoff of
        # tile kt overlaps exp(kt-1) instead of serializing after it (the
        # ring-2 WAR on sT allows exactly one tile of lookahead).
        sTs = {}

        def s_issue(kt, qcs=(0, 1)):
            if kt not in sTs:
                sTs[kt] = ps_s.tile([P, QL], f32, tag="s", name=f"sT_{b}_{h}_{kt}")
            s_matmul(sTs[kt], kT_b, b, h, kt, qcs=qcs)

        pTs = {}
        if split_first_exp:
            # fire the first exp as soon as the qc0 half of S exists --
            # shortens the DMA-serial ramp to the first ACT work
            pTs[0] = pp.tile([P, QL], bf, tag="pT", name=f"pT_{b}_{h}_0")
            s_issue(0, qcs=(0,))
            nc.scalar.activation(out=pTs[0][:, 0:512], in_=sTs[0][:, 0:512],
                                 func=Exp, scale=0.125)
            s_issue(0, qcs=(1,))
            nc.scalar.activation(out=pTs[0][:, 512:1024], in_=sTs[0][:, 512:1024],
                                 func=Exp, scale=0.125)
        else:
            s_issue(0)
        for kt in range(NKT):
            if kt + 1 < NKT:
                s_issue(kt + 1)
            if kt in pTs:
                pT = pTs.pop(kt)
            else:
                pT = pp.tile([P, QL], bf, tag="pT", name=f"pT_{b}_{h}_{kt}")
                nc.scalar.activation(out=pT[:], in_=sTs[kt][:], func=Exp, scale=0.125)
            del sTs[kt]
            if pT_prev is not None:
                av(kt - 1, pT_prev)
            pT_prev = pT
            for lump in lump_at.get(kt, ()):
                lump()
        av(NKT - 1, pT_prev)

    def norm_phase(b, h, o_ps, split=False):
        _mark(nc, f"norm{b}h{h}")
        # reciprocal of the denominator row (fp16: 0.05% err on a positive
        # denom), broadcast to 64 partitions with a rank-1 PE matmul
        # (ones[1,64]^T @ recip[1,QL]) -- keeps the normalize chain off the
        # DMA queue.
        f16 = mybir.dt.float16
        recip = npool.tile([1, QL], f16, tag="rc", name=f"rc_{b}_{h}")
        rb_sb = npool.tile([64, QL], f32, tag="rb", name=f"rb_{b}_{h}")
        ctmp = None
        if h == 1:
            ctmp = npool.tile([64, QL], bf, tag="ctmp", name=f"ct_{b}")
        # split=True runs the whole chain per token-half so the tail
        # out-projection can start on the first half early
        def half(hf, do_recip, do_mul):
            sl = slice(hf * 512, (hf + 1) * 512)
            if do_recip:
                rsl = sl if split else slice(0, QL)
                with nc.allow_low_precision(reason="fp16 recip of O(1e3) softmax denom"):
                    nc.vector.reciprocal(out=recip[:, rsl], in_=o_ps[64:65, rsl])
            rb_ps = ps_w.tile([64, 512], f32, tag="w", name=f"rb_{b}_{h}_{hf}")
            nc.tensor.matmul(rb_ps[:], ones_sb[:], recip[:, sl],
                             start=True, stop=True)
            nc.vector.tensor_copy(out=rb_sb[:, sl], in_=rb_ps[:])
            if not do_mul:
                return
            msl = sl if split else slice(0, QL)
            c0 = b * QL + msl.start
            c1 = b * QL + msl.stop
            if h == 0:
                nc.vector.tensor_mul(out=ctx_sb[0:64, c0:c1],
                                     in0=o_ps[0:64, msl], in1=rb_sb[:, msl])
            else:
                nc.vector.tensor_mul(out=ctmp[:, msl], in0=o_ps[0:64, msl],
                                     in1=rb_sb[:, msl])
                nc.sync.dma_start(out=ctx_sb[64:128, c0:c1], in_=ctmp[:, msl])

        half(0, do_recip=True, do_mul=split)
        half(1, do_recip=split, do_mul=True)

    # ---- emission timeline --------------------------------------------------
    # kv chunks pipeline into the phase that first consumes them (attention
    # reads kv tiles in kt order); each chunk's DMA lump runs one chunk ahead
    # of its projection lumps so PE never waits on the transfer. ACT-slack
    # phases (b0h1, b1h1) host the movable work: q-proj of b1, the first kv-b1
    # chunks, and the b0 out-projection.
    _mark(nc, "ramp")
    # First-exp critical path: wq + xq(qc0) -> qproj0, wk + a 256-token xkv
    # sub-chunk -> kT cols 0..255, then S(kt0,qc0)/exp fires (split_first_exp).
    qproj_half(0, 0)
    qproj_half(0, 1)
    xkv0_sub = []
    for sc in range(2):
        xst = xs.tile([P, KT_H, 256], bf, tag="x", name=f"xkv0s{sc}")
        nc.sync.dma_start(out=xst[:], in_=xkvT_r[:, :, sc * 256:(sc + 1) * 256])
        xkv0_sub.append(xst)
        pk = ps_w.tile([P, 256], f32, tag="w", name=f"pk00{sc}")
        for kt in range(KT_H):
            nc.tensor.matmul(
                pk[:], wk_sb[:, kt, :], xst[:, kt, :],
                start=(kt == 0), stop=(kt == KT_H - 1),
            )
        kT0 = kv_bufs[0][0]
        if S_FP8:
            nc.vector.tensor_copy(out=kT0[:, 0, sc * 256:(sc + 1) * 256], in_=pk[:])
            nc.vector.tensor_copy(out=kT0[:, 1, sc * 256:(sc + 1) * 256], in_=pk[:])
        else:
            nc.vector.tensor_copy(out=kT0[:, sc * 256:(sc + 1) * 256], in_=pk[:])
        if sc == 0:
            qproj_half(1, 0)
            qproj_half(1, 1)
            nc.sync.dma_start(
                out=wv_sb[:], in_=wv.rearrange("p (kt m) -> p kt m", kt=KT_H))

    def kv_v_sub(sc):
        xst = xkv0_sub[sc]
        v0 = kv_bufs[0][1]
        for mt in range(2):
            pv = ps_w.tile([P, P], f32, tag="w", name=f"pv00{sc}{mt}")
            for kt in range(KT_H):
                nc.tensor.matmul(
                    pv[:], xst[:, kt, mt * 128:(mt + 1) * 128], wv_sb[:, kt, :],
                    start=(kt == 0), stop=(kt == KT_H - 1),
                )
            ktile = sc * 2 + mt
            nc.vector.tensor_copy(out=v0[:, ktile, 0, 0:64], in_=pv[:, 0:64])
            nc.vector.tensor_copy(out=v0[:, ktile, 1, 0:64], in_=pv[:, 64:128])

    kv_v_sub(0)
    kv_dma(0, 1)

    def kv_sched(b, ch_from, lump_at=None):
        # deadline placement: chunk ch's K finishes by iteration 4ch-3 (< the
        # 4ch-2 limit imposed by the one-ahead S pre-issue), V by 4ch-1; the
        # chunk DMA is issued two chunks ahead so transfers never gate PE.
        la = lump_at if lump_at is not None else {}
        for ch in range(ch_from, NCH):
            base = max(4 * (ch - 1) - 2, 0)
            if ch + 1 < NCH:
                la.setdefault(max(base - 4, 0), []).append(
                    lambda ch=ch: kv_dma(b, ch + 1))
            la.setdefault(base, []).append(lambda ch=ch: kv_k_half(b, ch, 0))
            la.setdefault(base + 1, []).append(lambda ch=ch: kv_k_half(b, ch, 1))
            la.setdefault(base + 2, []).append(lambda ch=ch: kv_v_half(b, ch, 0))
            la.setdefault(base + 3, []).append(lambda ch=ch: kv_v_half(b, ch, 1))
        return la

    lumps_b0h0 = kv_sched(0, 1, {0: [lambda: kv_v_sub(1)]})
    lumps_b0h1 = [
        lambda: nc.sync.dma_start(out=wout_sb[:], in_=wout),
        lambda: qproj_half(2, 0), lambda: qproj_half(2, 1),
        lambda: qproj_half(3, 0), lambda: qproj_half(3, 1),
        lambda: kv_dma(1, 0),
        lambda: kv_k_half(1, 0, 0), lambda: kv_k_half(1, 0, 1),
        lambda: kv_v_half(1, 0, 0), lambda: kv_v_half(1, 0, 1),
        lambda: kv_dma(1, 1),
        lambda: kv_k_half(1, 1, 0), lambda: kv_k_half(1, 1, 1),
        lambda: kv_v_half(1, 1, 0), lambda: kv_v_half(1, 1, 1),
        lambda: kv_dma(1, 2),
    ]
    lumps_b1first = kv_sched(1, 2)
    lumps_b1second = [lambda mt=mt: outproj_tile(0, mt) for mt in range(QL // P)]

    # b1 runs h1 before h0 so the final normalize is the direct-write h0 path
    # (no ctx relocate DMA on the tail critical path)
    phases = [(0, 0), (0, 1), (1, 1), (1, 0)]
    phase_lumps = {(0, 0): lumps_b0h0, (0, 1): lumps_b0h1,
                   (1, 1): lumps_b1first, (1, 0): lumps_b1second}

    pre = ()
    o_prev = None
    bh_prev = None
    for b, h in phases:
        o_ps = ps_o.tile([P, QL], f32, tag="o", name=f"o_{b}_{h}")
        attn_phase(b, h, o_ps, phase_lumps[(b, h)], pre_lumps=pre,
                   split_first_exp=(b == 0 and h == 0))
        pre = ((lambda bb=b, hh=h, oo=o_ps: norm_phase(bb, hh, oo)),)
        o_prev, bh_prev = o_ps, (b, h)

    norm_phase(bh_prev[0], bh_prev[1], o_prev, split=True)
    _mark(nc, "tail")
    for mt in range(QL // P):
        outproj_tile(1, mt, tail=True)


def _build(reps=1):
    from contextlib import ExitStack

    import concourse.tile as tile
    from concourse import bacc, mybir

    f32 = mybir.dt.float32
    bf = mybir.dt.bfloat16

    nc = bacc.Bacc("TRN2", target_bir_lowering=False, debug=False,
                   num_devices=NCORES)
    aps = {
        "xqT": nc.dram_tensor("xqT", [H, TQ], bf, kind="ExternalInput").ap(),
        "xkvT": nc.dram_tensor("xkvT", [H, TK], bf, kind="ExternalInput").ap(),
        "wq": nc.dram_tensor("wq", [128, H], bf, kind="ExternalInput").ap(),
        "wk": nc.dram_tensor("wk", [128, H], bf, kind="ExternalInput").ap(),
        "wv": nc.dram_tensor("wv", [128, H], bf, kind="ExternalInput").ap(),
        "wout": nc.dram_tensor("wout", [128, H], bf, kind="ExternalInput").ap(),
        "out": nc.dram_tensor("out", [TQ, H], bf, kind="ExternalOutput").ap(),
    }
    with tile.TileContext(nc) as tc:
        with ExitStack() as ctx:
            pools = _make_pools(ctx, tc)
            for _ in range(reps):
                _emit(tc, aps, pools)
    nc.compile()
    return nc


def get_nc(reps=1):
    key = f"nc{reps}"
    if key not in _cache:
        _cache[key] = _build(reps)
    return _cache[key]


def make_in_maps(query, key_value, w_q, w_kv, w_out):
    import ml_dtypes
    cdt = ml_dtypes.bfloat16

    xq = np.asarray(query, np.float32).reshape(TQ, H)
    xkv = np.asarray(key_value, np.float32).reshape(TK, H)
    xqT = np.ascontiguousarray(xq.T).astype(cdt)
    xkvT = np.ascontiguousarray(xkv.T).astype(cdt)
    w_q = np.asarray(w_q, np.float32)
    w_kv = np.asarray(w_kv, np.float32)
    w_out = np.asarray(w_out, np.float32)

    def wprep(w):
        # [H, 128] -> [128p, KT_H*128m]: p is the within-k-tile partition so
        # the device-side DMA is line-contiguous per partition
        return np.ascontiguousarray(
            w.reshape(KT_H, 128, 128).transpose(1, 0, 2).reshape(128, H)
        ).astype(cdt)

    in_maps = []
    for c in range(NCORES):
        sl = slice(c * 128, (c + 1) * 128)
        in_maps.append({
            "xqT": xqT,
            "xkvT": xkvT,
            "wq": wprep(w_q[:, sl]),
            "wk": wprep(w_kv[:, sl]),
            "wv": wprep(w_kv[:, H + c * 128: H + (c + 1) * 128]),
            "wout": np.ascontiguousarray(w_out[sl, :]).astype(cdt),
        })
    return in_maps


LAST_EXEC_NS = None


def _run(in_maps, trace=False):
    global LAST_EXEC_NS
    from concourse import bass_utils

    nc = get_nc()
    res = bass_utils.run_bass_kernel_spmd(
        nc, in_maps, core_ids=list(range(NCORES)), trace=trace,
    )
    if res.exec_time_ns is not None:
        LAST_EXEC_NS = res.exec_time_ns
    return res


def kernel(query, key_value, w_q, w_kv, w_out):
    in_maps = make_in_maps(query, key_value, w_q, w_kv, w_out)
    res = _run(in_maps)
    total = np.zeros((TQ, H), np.float64)
    for c in range(NCORES):
        total += np.asarray(res.results[c]["out"], np.float64)
    return total.reshape(B, QL, H).astype(np.float32)
